# revision 41
# baseline (speedup 1.0000x reference)
"""DifferentialAttention (B=2, S=2048, D=2048, H=16, KVH=8) on 8 TRN2 NeuronCores.

Sharding: 8 cores = 2 (batch) x 4 (tensor-parallel head groups).
Core c = 4*b + r handles batch b and real heads 4r..4r+3:
  - column-parallel q/k/v projections (q heads 8r..8r+7, k heads 4r..4r+3,
    v heads 2r..2r+1), full causal differential attention for those heads,
  - row-parallel partial o_proj; host sums the 4 partials per batch.

Device math (per core), all fp32 data with float32r matmuls:
  - host passes x^T and W^T slices so every matmul contracts on partitions
  - scores computed transposed  S^T[k,q] = k . q  so exp -> AV needs no
    on-chip transposes
  - softmax without max subtraction (scores ~ N(0,1)); row sums and their
    partition broadcast fused into one all-ones matmul
  - RMS-norm folded:  out = u * rsqrt(mean_d(u^2) + eps*r1^2)  with
    u = O1 - (lam*r1/r2)*O2,  O = E@v unnormalized,  r = rowsum(E);
    subln weight and (1 - lambda_init) folded into Wo on the host;
    rsqrt computed as exp(-0.5*ln(x)) to stay on one ACT func table
  - causal masking: matmuls sliced to valid columns; 0/1 triangular mask
    multiplied into the single mixed 128x128 block per diagonal tile
  - DMA decongestion: x^T/weights packed into multi-block [128,2048]
    transfers, k/v weights resident in SBUF, RoPE rotate-half via DVE
    partition-shifted copies
"""

import math
import numpy as np

B, S, D = 2, 2048, 2048
H, KVH = 16, 8
Dh = 64
TP = 4
NCORES = 8
LAYER_IDX = 2
LAMBDA_INIT = 0.8 - 0.6 * math.exp(-0.3 * LAYER_IDX)
EPS = 1e-5
ROPE_THETA = 10000.0

_CACHE = {}


def _build_nc():
    import concourse.bass as bass  # noqa: F401
    import concourse.tile as tile
    from concourse import bacc, mybir

    F32 = mybir.dt.float32
    F32R = mybir.dt.float32r
    Act = mybir.ActivationFunctionType
    Alu = mybir.AluOpType

    nc = bacc.Bacc("TRN2", target_bir_lowering=False, debug=False)

    xT = nc.dram_tensor("xT", [D, S], F32R, kind="ExternalInput")
    wqT = nc.dram_tensor("wqT", [D, 512], F32R, kind="ExternalInput")
    wkT = nc.dram_tensor("wkT", [D, 256], F32R, kind="ExternalInput")
    wvT = nc.dram_tensor("wvT", [D, 256], F32R, kind="ExternalInput")
    woT = nc.dram_tensor("woT", [512, D], F32R, kind="ExternalInput")
    cosT_d = nc.dram_tensor("cosT", [128, S], F32, kind="ExternalInput")
    ssinT_d = nc.dram_tensor("ssinT", [128, S], F32, kind="ExternalInput")
    tri_d = nc.dram_tensor("tri", [128, 128], F32R, kind="ExternalInput")
    ones_d = nc.dram_tensor("ones", [128, 128], F32R, kind="ExternalInput")
    lam_d = nc.dram_tensor("lam", [128, 1], F32, kind="ExternalInput")
    out_d = nc.dram_tensor("out", [S, D], F32, kind="ExternalOutput")

    KD = D // 128  # 16 contraction tiles

    with tile.TileContext(nc) as tc:
        with tc.tile_pool(name="const", bufs=1) as constp, \
             tc.tile_pool(name="persist", bufs=1) as persist:

            cosT = constp.tile([128, S], F32, tag="cos")
            ssinT = constp.tile([128, S], F32, tag="ssin")
            tri = constp.tile([128, 128], F32R, tag="tri")
            ones = constp.tile([128, 128], F32R, tag="ones")
            lam = constp.tile([128, 1], F32, tag="lam")

            qT_sb = [persist.tile([128, S], F32R, tag=f"qT{m}", name=f"qT{m}")
                     for m in range(4)]
            kTd = [persist.tile([128, S], F32R, tag=f"kTd{h}", name=f"kTd{h}")
                   for h in range(4)]
            v_sb = [persist.tile([128, 256], F32R, tag=f"v{ms}", name=f"v{ms}")
                    for ms in range(16)]
            otf = [persist.tile([128, S], F32R, tag=f"otf{p}", name=f"otf{p}")
                   for p in range(4)]

            # ---------------- Phase A: projections + RoPE ----------------
            with tc.tile_pool(name="xtp", bufs=6) as xtp, \
                 tc.tile_pool(name="wstream", bufs=2) as wsp, \
                 tc.tile_pool(name="ropet", bufs=2) as rp, \
                 tc.tile_pool(name="psA", bufs=8, space="PSUM") as psA:

                def rope_core(ps, gc0):
                    """RoPE on a [128, 512] psum tile; returns (ra, rt) to add."""
                    gsl = slice(gc0, gc0 + 512)
                    qraw = rp.tile([128, 512], F32, tag="qraw", name="qraw")
                    nc.scalar.copy(qraw[:], ps[:])
                    qsw = rp.tile([128, 512], F32, tag="qsw", name="qsw")
                    for blk in range(4):
                        sb_ = (blk ^ 1) * 32
                        nc.vector.tensor_copy(
                            qsw[blk * 32:blk * 32 + 32, :], qraw[sb_:sb_ + 32, :])
                    nc.vector.tensor_mul(qraw[:], qraw[:], cosT[:, gsl])
                    nc.vector.tensor_mul(qsw[:], qsw[:], ssinT[:, gsl])
                    return qraw, qsw

                def rope_epilogue(ps, dst, gc0):
                    ra, rt = rope_core(ps, gc0)
                    nc.vector.tensor_add(dst[:, gc0:gc0 + 512], ra[:], rt[:])

                def rope_epilogue_kdup(ps, m, gc0):
                    """RoPE then duplicate each 64-row head half into kTd[2m+e]."""
                    gsl = slice(gc0, gc0 + 512)
                    ra, rt = rope_core(ps, gc0)
                    ktmp = rp.tile([128, 512], F32R, tag="ktmp", name="ktmp")
                    nc.vector.tensor_add(ktmp[:], ra[:], rt[:])
                    for e in range(2):
                        src = ktmp[e * 64:e * 64 + 64, :]
                        nc.sync.dma_start(out=kTd[2 * m + e][0:64, gsl], in_=src)
                        nc.sync.dma_start(out=kTd[2 * m + e][64:128, gsl], in_=src)

                # resident k/v weights, loaded once
                wk_r = []
                wv_r = []
                for kp in range(KD // 8):
                    t = wsp.tile([128, 2048], F32R, tag=f"wkr{kp}", name=f"wkr{kp}")
                    nc.sync.dma_start(
                        out=t[:].rearrange("p (eight n) -> p eight n", eight=8),
                        in_=wkT[kp * 1024:kp * 1024 + 1024, :]
                            .rearrange("(eight p) n -> p eight n", eight=8),
                    )
                    wk_r.append(t)
                    t = wsp.tile([128, 2048], F32R, tag=f"wvr{kp}", name=f"wvr{kp}")
                    nc.sync.dma_start(
                        out=t[:].rearrange("p (eight n) -> p eight n", eight=8),
                        in_=wvT[kp * 1024:kp * 1024 + 1024, :]
                            .rearrange("(eight p) n -> p eight n", eight=8),
                    )
                    wv_r.append(t)

                def wk_lhsT(kd, m):
                    return wk_r[kd // 8][:, (kd % 8) * 256 + m * 128:
                                         (kd % 8) * 256 + m * 128 + 128]

                def wv_rhs(kd):
                    return wv_r[kd // 8][:, (kd % 8) * 256:(kd % 8) * 256 + 256]

                for sh in range(4):
                    c0 = 512 * sh
                    # x^T tiles: two 128-row blocks packed along free dim
                    xt4 = []
                    for kp in range(KD // 4):
                        t = xtp.tile([128, 2048], F32R, tag="xt", name=f"xt{kp}")
                        nc.sync.dma_start(
                            out=t[:].rearrange("p (four n) -> p four n", four=4),
                            in_=xT[kp * 512:kp * 512 + 512, c0:c0 + 512]
                                .rearrange("(four p) n -> p four n", four=4),
                        )
                        xt4.append(t)

                    def xt_rhs(kd):
                        return xt4[kd // 4][:, (kd % 4) * 512:(kd % 4) * 512 + 512]

                    # q projection: qT[j, s] for j in [0,512)
                    qps = {}
                    for kp in range(KD // 4):
                        wqb = wsp.tile([128, 2048], F32R, tag="wq", name="wqb")
                        nc.sync.dma_start(
                            out=wqb[:].rearrange("p (four n) -> p four n", four=4),
                            in_=wqT[kp * 512:kp * 512 + 512, :]
                                .rearrange("(four p) n -> p four n", four=4),
                        )
                        for t in range(4):
                            kd = kp * 4 + t
                            for m in range(4):
                                if kd == 0:
                                    qps[m] = psA.tile([128, 512], F32, tag="pa",
                                                      name=f"qps{m}")
                                nc.tensor.matmul(
                                    qps[m][:],
                                    wqb[:, t * 512 + m * 128:t * 512 + m * 128 + 128],
                                    xt_rhs(kd),
                                    start=(kd == 0), stop=(kd == KD - 1),
                                )
                    if sh == 0:
                        load_tables_and_kv()
                    for m in range(4):
                        rope_epilogue(qps[m], qT_sb[m], c0)

                    # k projection (4 blocks packed per DMA)
                    kps = {}
                    for kd in range(KD):
                        for m in range(2):
                            if kd == 0:
                                kps[m] = psA.tile([128, 512], F32, tag="pa",
                                                  name=f"kps{m}")
                            nc.tensor.matmul(
                                kps[m][:],
                                wk_lhsT(kd, m),
                                xt_rhs(kd),
                                start=(kd == 0), stop=(kd == KD - 1),
                            )
                    for m in range(2):
                        rope_epilogue_kdup(kps[m], m, c0)

                    # v projection: v[s, j] (s on partitions)
                    vps = {}
                    for kd in range(KD):
                        for ms in range(4):
                            if kd == 0:
                                vps[ms] = psA.tile([128, 512], F32, tag="pa",
                                                   name=f"vps{ms}")
                            nc.tensor.matmul(
                                vps[ms][:, 0:256],
                                xt_rhs(kd)[:, ms * 128:ms * 128 + 128],
                                wv_rhs(kd),
                                start=(kd == 0), stop=(kd == KD - 1),
                            )
                    for ms in range(4):
                        nc.scalar.copy(v_sb[sh * 4 + ms][:], vps[ms][:, 0:256])

            # ---------------- Phase B: attention ----------------
            with tc.tile_pool(name="etp", bufs=8) as etp, \
                 tc.tile_pool(name="ebp", bufs=8) as ebp, \
                 tc.tile_pool(name="psAcc", bufs=4, space="PSUM") as psAcc, \
                 tc.tile_pool(name="psS", bufs=4, space="PSUM") as psS:

                for p, qi in [(pp, qq) for pp in range(4) for qq in range(4)]:
                    if True:
                        vh = p // 2
                        q0 = 512 * qi
                        nki = 4 * qi + 4
                        OT1 = psAcc.tile([128, 512], F32, tag="acc", name="OT1")
                        OT2 = psAcc.tile([128, 512], F32, tag="acc", name="OT2")
                        R1 = psAcc.tile([128, 512], F32, tag="acc", name="R1")
                        R2 = psAcc.tile([128, 512], F32, tag="acc", name="R2")
                        for ki in range(nki):
                            j = ki - 4 * qi
                            vc = 128 * j if j > 0 else 0
                            ksl = slice(ki * 128, ki * 128 + 128)
                            st, sp_ = (ki == 0), (ki == nki - 1)
                            S1 = psS.tile([128, 512], F32, tag="s", name="S1")
                            S2 = psS.tile([128, 512], F32, tag="s", name="S2")
                            nc.tensor.matmul(
                                S1[:, vc:512],
                                kTd[p][0:64, ksl],
                                qT_sb[p][0:64, q0 + vc:q0 + 512],
                                start=True, stop=True,
                            )
                            nc.tensor.matmul(
                                S2[:, vc:512],
                                kTd[p][64:128, ksl],
                                qT_sb[p][64:128, q0 + vc:q0 + 512],
                                start=True, stop=True,
                            )
                            ET1 = etp.tile([128, 512], F32R, tag="e1", name="ET1")
                            ET2 = etp.tile([128, 512], F32R, tag="e2", name="ET2")
                            nc.scalar.activation(ET1[:, vc:512], S1[:, vc:512], Act.Exp)
                            nc.scalar.activation(ET2[:, vc:512], S2[:, vc:512], Act.Exp)
                            if j >= 0:  # diagonal tile: mask mixed 128x128 block
                                msl = slice(vc, vc + 128)
                                nc.vector.tensor_mul(ET1[:, msl], ET1[:, msl], tri[:])
                                nc.vector.tensor_mul(ET2[:, msl], ET2[:, msl], tri[:])
                            vt = v_sb[ki][:, vh * 128:vh * 128 + 128]
                            nc.tensor.matmul(OT1[:, vc:512], vt, ET1[:, vc:512],
                                             start=st, stop=sp_)
                            nc.tensor.matmul(R1[:, vc:512], ones[:], ET1[:, vc:512],
                                             start=st, stop=sp_)
                            nc.tensor.matmul(OT2[:, vc:512], vt, ET2[:, vc:512],
                                             start=st, stop=sp_)
                            nc.tensor.matmul(R2[:, vc:512], ones[:], ET2[:, vc:512],
                                             start=st, stop=sp_)

                        # epilogue: normalize + differential combine + RMS
                        rcp2 = ebp.tile([128, 512], F32, tag="eb", name="rcp2")
                        nc.vector.reciprocal(rcp2[:], R2[:])
                        mb = ebp.tile([128, 512], F32, tag="eb", name="mb")
                        nc.vector.scalar_tensor_tensor(
                            mb[:], R1[:], lam[:, 0:1], rcp2[:], Alu.mult, Alu.mult)
                        r1e = ebp.tile([128, 512], F32, tag="eb", name="r1e")
                        nc.vector.tensor_scalar_mul(r1e[:], R1[:], math.sqrt(EPS))
                        tt = ebp.tile([128, 512], F32, tag="eb", name="tt")
                        nc.vector.tensor_mul(tt[:], OT2[:], mb[:])
                        u = ebp.tile([128, 512], F32, tag="eb", name="u")
                        nc.vector.tensor_sub(u[:], OT1[:], tt[:])
                        sq = ebp.tile([128, 512], F32R, tag="eb", name="sq")
                        nc.vector.tensor_mul(sq[:], u[:], u[:])
                        varp = psAcc.tile([128, 512], F32, tag="acc", name="varp")
                        nc.tensor.matmul(varp[:], ones[:], sq[:], start=True, stop=True)
                        t2 = ebp.tile([128, 512], F32, tag="eb", name="t2")
                        nc.vector.tensor_mul(t2[:], r1e[:], r1e[:])
                        pre = ebp.tile([128, 512], F32, tag="eb", name="pre")
                        nc.vector.scalar_tensor_tensor(
                            pre[:], varp[:], 1.0 / 128.0, t2[:], Alu.mult, Alu.add)
                        lnp = ebp.tile([128, 512], F32, tag="eb", name="lnp")
                        nc.scalar.activation(lnp[:], pre[:], Act.Ln)
                        sf = ebp.tile([128, 512], F32, tag="eb", name="sf")
                        nc.scalar.activation(sf[:], lnp[:], Act.Exp, scale=-0.5)
                        nc.vector.tensor_mul(otf[p][:, q0:q0 + 512], u[:], sf[:])

            # ---------------- Phase C: o_proj (row-parallel partial) -------
            with tc.tile_pool(name="wop", bufs=1) as wop, \
                 tc.tile_pool(name="outp", bufs=2) as outp, \
                 tc.tile_pool(name="psC", bufs=4, space="PSUM") as psC:
                wo_t = {}
                for n in range(4):
                    t = wop.tile([128, 2048], F32R, tag=f"wo{n}", name=f"wo{n}")
                    nc.sync.dma_start(
                        out=t[:].rearrange("p (four n) -> p four n", four=4),
                        in_=woT[:, n * 512:n * 512 + 512]
                            .rearrange("(four p) n -> p four n", four=4),
                    )
                    for kc in range(4):
                        wo_t[n, kc] = t[:, kc * 512:kc * 512 + 512]
                for m in range(16):
                    osb = outp.tile([128, 2048], F32, tag="ob", name="osb")
                    for n in range(4):
                        ps = psC.tile([128, 512], F32, tag="pc", name="pc")
                        for kc in range(4):
                            nc.tensor.matmul(
                                ps[:],
                                otf[kc][:, m * 128:m * 128 + 128],
                                wo_t[n, kc],
                                start=(kc == 0), stop=(kc == 3),
                            )
                        nc.vector.tensor_copy(osb[:, n * 512:n * 512 + 512], ps[:])
                    nc.sync.dma_start(out=out_d[m * 128:m * 128 + 128, :], in_=osb[:])

    nc.compile()
    return nc


def _host_tables():
    inv = ROPE_THETA ** (-np.arange(Dh, dtype=np.float64) / Dh)
    pos = np.arange(S, dtype=np.float64)
    fr = pos[:, None] * inv[None, :]              # [S, 64]
    cos = np.cos(fr).astype(np.float32)           # [S, 64]
    sin = np.sin(fr).astype(np.float32)
    d = np.arange(128) % 64
    cosT = cos[:, d].T.copy()                     # [128, S]
    sgn = np.where((np.arange(128) % 64) < 32, -1.0, 1.0).astype(np.float32)
    ssinT = (sin[:, d].T * sgn[:, None]).copy()
    tri = np.triu(np.ones((128, 128), np.float32))  # tri[k, q] = 1 if q >= k
    ones = np.ones((128, 128), np.float32)
    return np.ascontiguousarray(cosT), np.ascontiguousarray(ssinT), tri, ones


def kernel(hidden_states, Wq, Wk, Wv, Wo,
           lambda_q1, lambda_k1, lambda_q2, lambda_k2, subln_weight):
    from concourse.bass_utils import run_bass_kernel_spmd

    if "nc" not in _CACHE:
        _CACHE["nc"] = _build_nc()
        _CACHE["tables"] = _host_tables()
    nc = _CACHE["nc"]
    cosT, ssinT, tri, ones = _CACHE["tables"]

    f32 = np.float32
    hs = np.asarray(hidden_states, f32)
    Wq = np.asarray(Wq, f32)
    Wk = np.asarray(Wk, f32)
    Wv = np.asarray(Wv, f32)
    Wo = np.asarray(Wo, f32)
    subln = np.asarray(subln_weight, f32)

    lam1 = np.exp(np.sum(np.asarray(lambda_q1, f32) * np.asarray(lambda_k1, f32),
                         dtype=f32))
    lam2 = np.exp(np.sum(np.asarray(lambda_q2, f32) * np.asarray(lambda_k2, f32),
                         dtype=f32))
    lam_full = f32(lam1 - lam2 + LAMBDA_INIT)
    lam_arr = np.full((128, 1), lam_full, f32)

    scale = f32(Dh ** -0.5)
    wprime = (np.tile(subln, H) * f32(1.0 - LAMBDA_INIT)).astype(f32)  # [2048]
    WoS = Wo * wprime[None, :]

    in_maps = []
    for c in range(NCORES):
        b, r = c // TP, c % TP
        in_maps.append({
            "xT": np.ascontiguousarray(hs[b].T),
            "wqT": np.ascontiguousarray((Wq[512 * r:512 * r + 512, :] * scale).T),
            "wkT": np.ascontiguousarray(Wk[256 * r:256 * r + 256, :].T),
            "wvT": np.ascontiguousarray(Wv[256 * r:256 * r + 256, :].T),
            "woT": np.ascontiguousarray(WoS[:, 512 * r:512 * r + 512].T),
            "cosT": cosT, "ssinT": ssinT, "tri": tri, "ones": ones,
            "lam": lam_arr,
        })

    res = run_bass_kernel_spmd(nc, in_maps, core_ids=list(range(NCORES)))
    out = np.zeros((B, S, D), f32)
    for c in range(NCORES):
        out[c // TP] += res.results[c]["out"]
    return out


# revision 42
# speedup vs baseline: 1.0077x; 1.0077x over previous
"""DifferentialAttention (B=2, S=2048, D=2048, H=16, KVH=8) on 8 TRN2 NeuronCores.

Sharding: 8 cores = 2 (batch) x 4 (tensor-parallel head groups).
Core c = 4*b + r handles batch b and real heads 4r..4r+3:
  - column-parallel q/k/v projections (q heads 8r..8r+7, k heads 4r..4r+3,
    v heads 2r..2r+1), full causal differential attention for those heads,
  - row-parallel partial o_proj; host sums the 4 partials per batch.

Device math (per core), all fp32 data with float32r matmuls:
  - host passes x^T and W^T slices so every matmul contracts on partitions
  - scores computed transposed  S^T[k,q] = k . q  so exp -> AV needs no
    on-chip transposes
  - softmax without max subtraction (scores ~ N(0,1)); row sums and their
    partition broadcast fused into one all-ones matmul
  - RMS-norm folded:  out = u * rsqrt(mean_d(u^2) + eps*r1^2)  with
    u = O1 - (lam*r1/r2)*O2,  O = E@v unnormalized,  r = rowsum(E);
    subln weight and (1 - lambda_init) folded into Wo on the host;
    rsqrt computed as exp(-0.5*ln(x)) to stay on one ACT func table
  - causal masking: matmuls sliced to valid columns; 0/1 triangular mask
    multiplied into the single mixed 128x128 block per diagonal tile
  - DMA decongestion: x^T/weights packed into multi-block [128,2048]
    transfers, k/v weights resident in SBUF, RoPE rotate-half via DVE
    partition-shifted copies
"""

import math
import numpy as np

B, S, D = 2, 2048, 2048
H, KVH = 16, 8
Dh = 64
TP = 4
NCORES = 8
LAYER_IDX = 2
LAMBDA_INIT = 0.8 - 0.6 * math.exp(-0.3 * LAYER_IDX)
EPS = 1e-5
ROPE_THETA = 10000.0

_CACHE = {}


def _build_nc():
    import concourse.bass as bass  # noqa: F401
    import concourse.tile as tile
    from concourse import bacc, mybir

    F32 = mybir.dt.float32
    F32R = mybir.dt.float32r
    Act = mybir.ActivationFunctionType
    Alu = mybir.AluOpType

    nc = bacc.Bacc("TRN2", target_bir_lowering=False, debug=False)

    xT = nc.dram_tensor("xT", [D, S], F32R, kind="ExternalInput")
    wqT = nc.dram_tensor("wqT", [D, 512], F32R, kind="ExternalInput")
    wkT = nc.dram_tensor("wkT", [D, 256], F32R, kind="ExternalInput")
    wvT = nc.dram_tensor("wvT", [D, 256], F32R, kind="ExternalInput")
    woT = nc.dram_tensor("woT", [512, D], F32R, kind="ExternalInput")
    cosT_d = nc.dram_tensor("cosT", [128, S], F32, kind="ExternalInput")
    ssinT_d = nc.dram_tensor("ssinT", [128, S], F32, kind="ExternalInput")
    tri_d = nc.dram_tensor("tri", [128, 128], F32R, kind="ExternalInput")
    ones_d = nc.dram_tensor("ones", [128, 128], F32R, kind="ExternalInput")
    lam_d = nc.dram_tensor("lam", [128, 1], F32, kind="ExternalInput")
    out_d = nc.dram_tensor("out", [S, D], F32, kind="ExternalOutput")

    KD = D // 128  # 16 contraction tiles

    with tile.TileContext(nc) as tc:
        with tc.tile_pool(name="const", bufs=1) as constp, \
             tc.tile_pool(name="persist", bufs=1) as persist:

            cosT = constp.tile([128, S], F32, tag="cos")
            ssinT = constp.tile([128, S], F32, tag="ssin")
            tri = constp.tile([128, 128], F32R, tag="tri")
            ones = constp.tile([128, 128], F32R, tag="ones")
            lam = constp.tile([128, 1], F32, tag="lam")

            qT_sb = [persist.tile([128, S], F32R, tag=f"qT{m}", name=f"qT{m}")
                     for m in range(4)]
            kTd = [persist.tile([128, S], F32R, tag=f"kTd{h}", name=f"kTd{h}")
                   for h in range(4)]
            v_sb = [persist.tile([128, 256], F32R, tag=f"v{ms}", name=f"v{ms}")
                    for ms in range(16)]
            otf = [persist.tile([128, S], F32R, tag=f"otf{p}", name=f"otf{p}")
                   for p in range(4)]

            # ---------------- Phase A: projections + RoPE ----------------
            with tc.tile_pool(name="xtp", bufs=6) as xtp, \
                 tc.tile_pool(name="wstream", bufs=2) as wsp, \
                 tc.tile_pool(name="ropet", bufs=2) as rp, \
                 tc.tile_pool(name="psA", bufs=8, space="PSUM") as psA:

                def rope_core(ps, gc0):
                    """RoPE on a [128, 512] psum tile; returns (ra, rt) to add."""
                    gsl = slice(gc0, gc0 + 512)
                    qraw = rp.tile([128, 512], F32, tag="qraw", name="qraw")
                    nc.scalar.copy(qraw[:], ps[:])
                    qsw = rp.tile([128, 512], F32, tag="qsw", name="qsw")
                    for blk in range(4):
                        sb_ = (blk ^ 1) * 32
                        nc.vector.tensor_copy(
                            qsw[blk * 32:blk * 32 + 32, :], qraw[sb_:sb_ + 32, :])
                    nc.vector.tensor_mul(qraw[:], qraw[:], cosT[:, gsl])
                    nc.vector.tensor_mul(qsw[:], qsw[:], ssinT[:, gsl])
                    return qraw, qsw

                def rope_epilogue(ps, dst, gc0):
                    ra, rt = rope_core(ps, gc0)
                    nc.vector.tensor_add(dst[:, gc0:gc0 + 512], ra[:], rt[:])

                def rope_epilogue_kdup(ps, m, gc0):
                    """RoPE then duplicate each 64-row head half into kTd[2m+e]."""
                    gsl = slice(gc0, gc0 + 512)
                    ra, rt = rope_core(ps, gc0)
                    ktmp = rp.tile([128, 512], F32R, tag="ktmp", name="ktmp")
                    nc.vector.tensor_add(ktmp[:], ra[:], rt[:])
                    for e in range(2):
                        src = ktmp[e * 64:e * 64 + 64, :]
                        nc.sync.dma_start(out=kTd[2 * m + e][0:64, gsl], in_=src)
                        nc.sync.dma_start(out=kTd[2 * m + e][64:128, gsl], in_=src)

                # resident k/v weights, loaded once
                wk_r = []
                wv_r = []
                for kp in range(KD // 8):
                    t = wsp.tile([128, 2048], F32R, tag=f"wkr{kp}", name=f"wkr{kp}")
                    nc.sync.dma_start(
                        out=t[:].rearrange("p (eight n) -> p eight n", eight=8),
                        in_=wkT[kp * 1024:kp * 1024 + 1024, :]
                            .rearrange("(eight p) n -> p eight n", eight=8),
                    )
                    wk_r.append(t)
                    t = wsp.tile([128, 2048], F32R, tag=f"wvr{kp}", name=f"wvr{kp}")
                    nc.sync.dma_start(
                        out=t[:].rearrange("p (eight n) -> p eight n", eight=8),
                        in_=wvT[kp * 1024:kp * 1024 + 1024, :]
                            .rearrange("(eight p) n -> p eight n", eight=8),
                    )
                    wv_r.append(t)

                def wk_lhsT(kd, m):
                    return wk_r[kd // 8][:, (kd % 8) * 256 + m * 128:
                                         (kd % 8) * 256 + m * 128 + 128]

                def wv_rhs(kd):
                    return wv_r[kd // 8][:, (kd % 8) * 256:(kd % 8) * 256 + 256]

                for sh in range(4):
                    c0 = 512 * sh
                    # x^T tiles: two 128-row blocks packed along free dim
                    xt4 = []
                    for kp in range(KD // 4):
                        t = xtp.tile([128, 2048], F32R, tag="xt", name=f"xt{kp}")
                        nc.sync.dma_start(
                            out=t[:].rearrange("p (four n) -> p four n", four=4),
                            in_=xT[kp * 512:kp * 512 + 512, c0:c0 + 512]
                                .rearrange("(four p) n -> p four n", four=4),
                        )
                        xt4.append(t)

                    def xt_rhs(kd):
                        return xt4[kd // 4][:, (kd % 4) * 512:(kd % 4) * 512 + 512]

                    # q projection: qT[j, s] for j in [0,512)
                    qps = {}
                    for kp in range(KD // 4):
                        wqb = wsp.tile([128, 2048], F32R, tag="wq", name="wqb")
                        nc.sync.dma_start(
                            out=wqb[:].rearrange("p (four n) -> p four n", four=4),
                            in_=wqT[kp * 512:kp * 512 + 512, :]
                                .rearrange("(four p) n -> p four n", four=4),
                        )
                        for t in range(4):
                            kd = kp * 4 + t
                            for m in range(4):
                                if kd == 0:
                                    qps[m] = psA.tile([128, 512], F32, tag="pa",
                                                      name=f"qps{m}")
                                nc.tensor.matmul(
                                    qps[m][:],
                                    wqb[:, t * 512 + m * 128:t * 512 + m * 128 + 128],
                                    xt_rhs(kd),
                                    start=(kd == 0), stop=(kd == KD - 1),
                                )
                    if sh == 0:
                        load_tables_and_kv()
                    for m in range(4):
                        rope_epilogue(qps[m], qT_sb[m], c0)

                    # k projection (4 blocks packed per DMA)
                    kps = {}
                    for kd in range(KD):
                        for m in range(2):
                            if kd == 0:
                                kps[m] = psA.tile([128, 512], F32, tag="pa",
                                                  name=f"kps{m}")
                            nc.tensor.matmul(
                                kps[m][:],
                                wk_lhsT(kd, m),
                                xt_rhs(kd),
                                start=(kd == 0), stop=(kd == KD - 1),
                            )
                    for m in range(2):
                        rope_epilogue_kdup(kps[m], m, c0)

                    # v projection: v[s, j] (s on partitions)
                    vps = {}
                    for kd in range(KD):
                        for ms in range(4):
                            if kd == 0:
                                vps[ms] = psA.tile([128, 512], F32, tag="pa",
                                                   name=f"vps{ms}")
                            nc.tensor.matmul(
                                vps[ms][:, 0:256],
                                xt_rhs(kd)[:, ms * 128:ms * 128 + 128],
                                wv_rhs(kd),
                                start=(kd == 0), stop=(kd == KD - 1),
                            )
                    for ms in range(4):
                        nc.scalar.copy(v_sb[sh * 4 + ms][:], vps[ms][:, 0:256])

            # ---------------- Phase B: attention ----------------
            with tc.tile_pool(name="etp", bufs=8) as etp, \
                 tc.tile_pool(name="ebp", bufs=8) as ebp, \
                 tc.tile_pool(name="psAcc", bufs=4, space="PSUM") as psAcc, \
                 tc.tile_pool(name="psS", bufs=4, space="PSUM") as psS:

                for p, qi in [(pp, qq) for pp in range(4) for qq in range(4)]:
                    if True:
                        vh = p // 2
                        q0 = 512 * qi
                        nki = 4 * qi + 4
                        OT1 = psAcc.tile([128, 512], F32, tag="acc", name="OT1")
                        OT2 = psAcc.tile([128, 512], F32, tag="acc", name="OT2")
                        R1 = psAcc.tile([128, 512], F32, tag="acc", name="R1")
                        R2 = psAcc.tile([128, 512], F32, tag="acc", name="R2")
                        for ki in list(range(4 * qi, nki)) + list(range(4 * qi)):
                            j = ki - 4 * qi
                            vc = 128 * j if j > 0 else 0
                            ksl = slice(ki * 128, ki * 128 + 128)
                            st = (ki == 4 * qi)
                            sp_ = (ki == 4 * qi - 1) if qi > 0 else (ki == nki - 1)
                            S1 = psS.tile([128, 512], F32, tag="s", name="S1")
                            S2 = psS.tile([128, 512], F32, tag="s", name="S2")
                            nc.tensor.matmul(
                                S1[:, vc:512],
                                kTd[p][0:64, ksl],
                                qT_sb[p][0:64, q0 + vc:q0 + 512],
                                start=True, stop=True,
                            )
                            nc.tensor.matmul(
                                S2[:, vc:512],
                                kTd[p][64:128, ksl],
                                qT_sb[p][64:128, q0 + vc:q0 + 512],
                                start=True, stop=True,
                            )
                            ET1 = etp.tile([128, 512], F32R, tag="e1", name="ET1")
                            ET2 = etp.tile([128, 512], F32R, tag="e2", name="ET2")
                            nc.scalar.activation(ET1[:, vc:512], S1[:, vc:512], Act.Exp)
                            nc.scalar.activation(ET2[:, vc:512], S2[:, vc:512], Act.Exp)
                            if j >= 0:  # diagonal tile: mask mixed 128x128 block
                                msl = slice(vc, vc + 128)
                                nc.vector.tensor_mul(ET1[:, msl], ET1[:, msl], tri[:])
                                nc.vector.tensor_mul(ET2[:, msl], ET2[:, msl], tri[:])
                            vt = v_sb[ki][:, vh * 128:vh * 128 + 128]
                            nc.tensor.matmul(OT1[:, vc:512], vt, ET1[:, vc:512],
                                             start=st, stop=sp_)
                            nc.tensor.matmul(R1[:, vc:512], ones[:], ET1[:, vc:512],
                                             start=st, stop=sp_)
                            nc.tensor.matmul(OT2[:, vc:512], vt, ET2[:, vc:512],
                                             start=st, stop=sp_)
                            nc.tensor.matmul(R2[:, vc:512], ones[:], ET2[:, vc:512],
                                             start=st, stop=sp_)

                        # epilogue: normalize + differential combine + RMS
                        rcp2 = ebp.tile([128, 512], F32, tag="eb", name="rcp2")
                        nc.vector.reciprocal(rcp2[:], R2[:])
                        mb = ebp.tile([128, 512], F32, tag="eb", name="mb")
                        nc.vector.scalar_tensor_tensor(
                            mb[:], R1[:], lam[:, 0:1], rcp2[:], Alu.mult, Alu.mult)
                        r1e = ebp.tile([128, 512], F32, tag="eb", name="r1e")
                        nc.vector.tensor_scalar_mul(r1e[:], R1[:], math.sqrt(EPS))
                        tt = ebp.tile([128, 512], F32, tag="eb", name="tt")
                        nc.vector.tensor_mul(tt[:], OT2[:], mb[:])
                        u = ebp.tile([128, 512], F32, tag="eb", name="u")
                        nc.vector.tensor_sub(u[:], OT1[:], tt[:])
                        sq = ebp.tile([128, 512], F32R, tag="eb", name="sq")
                        nc.vector.tensor_mul(sq[:], u[:], u[:])
                        varp = psAcc.tile([128, 512], F32, tag="acc", name="varp")
                        nc.tensor.matmul(varp[:], ones[:], sq[:], start=True, stop=True)
                        t2 = ebp.tile([128, 512], F32, tag="eb", name="t2")
                        nc.vector.tensor_mul(t2[:], r1e[:], r1e[:])
                        pre = ebp.tile([128, 512], F32, tag="eb", name="pre")
                        nc.vector.scalar_tensor_tensor(
                            pre[:], varp[:], 1.0 / 128.0, t2[:], Alu.mult, Alu.add)
                        lnp = ebp.tile([128, 512], F32, tag="eb", name="lnp")
                        nc.scalar.activation(lnp[:], pre[:], Act.Ln)
                        sf = ebp.tile([128, 512], F32, tag="eb", name="sf")
                        nc.scalar.activation(sf[:], lnp[:], Act.Exp, scale=-0.5)
                        nc.vector.tensor_mul(otf[p][:, q0:q0 + 512], u[:], sf[:])

            # ---------------- Phase C: o_proj (row-parallel partial) -------
            with tc.tile_pool(name="wop", bufs=1) as wop, \
                 tc.tile_pool(name="outp", bufs=2) as outp, \
                 tc.tile_pool(name="psC", bufs=4, space="PSUM") as psC:
                wo_t = {}
                for n in range(4):
                    t = wop.tile([128, 2048], F32R, tag=f"wo{n}", name=f"wo{n}")
                    nc.sync.dma_start(
                        out=t[:].rearrange("p (four n) -> p four n", four=4),
                        in_=woT[:, n * 512:n * 512 + 512]
                            .rearrange("(four p) n -> p four n", four=4),
                    )
                    for kc in range(4):
                        wo_t[n, kc] = t[:, kc * 512:kc * 512 + 512]
                for m in range(16):
                    osb = outp.tile([128, 2048], F32, tag="ob", name="osb")
                    for n in range(4):
                        ps = psC.tile([128, 512], F32, tag="pc", name="pc")
                        for kc in range(4):
                            nc.tensor.matmul(
                                ps[:],
                                otf[kc][:, m * 128:m * 128 + 128],
                                wo_t[n, kc],
                                start=(kc == 0), stop=(kc == 3),
                            )
                        nc.vector.tensor_copy(osb[:, n * 512:n * 512 + 512], ps[:])
                    nc.sync.dma_start(out=out_d[m * 128:m * 128 + 128, :], in_=osb[:])

    nc.compile()
    return nc


def _host_tables():
    inv = ROPE_THETA ** (-np.arange(Dh, dtype=np.float64) / Dh)
    pos = np.arange(S, dtype=np.float64)
    fr = pos[:, None] * inv[None, :]              # [S, 64]
    cos = np.cos(fr).astype(np.float32)           # [S, 64]
    sin = np.sin(fr).astype(np.float32)
    d = np.arange(128) % 64
    cosT = cos[:, d].T.copy()                     # [128, S]
    sgn = np.where((np.arange(128) % 64) < 32, -1.0, 1.0).astype(np.float32)
    ssinT = (sin[:, d].T * sgn[:, None]).copy()
    tri = np.triu(np.ones((128, 128), np.float32))  # tri[k, q] = 1 if q >= k
    ones = np.ones((128, 128), np.float32)
    return np.ascontiguousarray(cosT), np.ascontiguousarray(ssinT), tri, ones


def kernel(hidden_states, Wq, Wk, Wv, Wo,
           lambda_q1, lambda_k1, lambda_q2, lambda_k2, subln_weight):
    from concourse.bass_utils import run_bass_kernel_spmd

    if "nc" not in _CACHE:
        _CACHE["nc"] = _build_nc()
        _CACHE["tables"] = _host_tables()
    nc = _CACHE["nc"]
    cosT, ssinT, tri, ones = _CACHE["tables"]

    f32 = np.float32
    hs = np.asarray(hidden_states, f32)
    Wq = np.asarray(Wq, f32)
    Wk = np.asarray(Wk, f32)
    Wv = np.asarray(Wv, f32)
    Wo = np.asarray(Wo, f32)
    subln = np.asarray(subln_weight, f32)

    lam1 = np.exp(np.sum(np.asarray(lambda_q1, f32) * np.asarray(lambda_k1, f32),
                         dtype=f32))
    lam2 = np.exp(np.sum(np.asarray(lambda_q2, f32) * np.asarray(lambda_k2, f32),
                         dtype=f32))
    lam_full = f32(lam1 - lam2 + LAMBDA_INIT)
    lam_arr = np.full((128, 1), lam_full, f32)

    scale = f32(Dh ** -0.5)
    wprime = (np.tile(subln, H) * f32(1.0 - LAMBDA_INIT)).astype(f32)  # [2048]
    WoS = Wo * wprime[None, :]

    in_maps = []
    for c in range(NCORES):
        b, r = c // TP, c % TP
        in_maps.append({
            "xT": np.ascontiguousarray(hs[b].T),
            "wqT": np.ascontiguousarray((Wq[512 * r:512 * r + 512, :] * scale).T),
            "wkT": np.ascontiguousarray(Wk[256 * r:256 * r + 256, :].T),
            "wvT": np.ascontiguousarray(Wv[256 * r:256 * r + 256, :].T),
            "woT": np.ascontiguousarray(WoS[:, 512 * r:512 * r + 512].T),
            "cosT": cosT, "ssinT": ssinT, "tri": tri, "ones": ones,
            "lam": lam_arr,
        })

    res = run_bass_kernel_spmd(nc, in_maps, core_ids=list(range(NCORES)))
    out = np.zeros((B, S, D), f32)
    for c in range(NCORES):
        out[c // TP] += res.results[c]["out"]
    return out
